# revision 24
# baseline (speedup 1.0000x reference)
"""Trainium2 Bass kernel for nn_AutoformerBase (sparse_attention).

Algorithm (algebraically reduced from the reference):
  mean_value[b, tau] = sum_{t,j} rho[b,t,j] * k_in[b,(t-tau)%L,j],
    where rho = q_in @ A and A = (Wq @ Wk^T)/D   (bq/bk only shift
    mean_value by a per-batch constant -> no effect on top-k or softmax).
  index = top6 of mean over b of mean_value  (one tiny AllReduce)
  tmp_corr = softmax(mean_value[:, index])
  out = sum_k tmp_corr[:,k] * Z[:, (t+tau_k)%L, :] + (bv@Wo + bo),
    where Z = v_in @ (Wv @ Wo)  (rolls commute with the right matmul;
    softmax weights sum to 1 for the bias term).

v2 design (per core, data-parallel over batch, all operands bf16):
  - circulant-H Gram: H[p,c] = sum_{tr,j} kT[j,128tr+p] rho[j,(128tr+c)%L]
    accumulated over all (tr,j) into ONE [128,1024] PSUM tile (the per-tr
    shift is absorbed into static wrap-split reads of rho), so only one
    sheared SBUF spill per batch and no identity-fold matmuls.
    mv[tau] = sum_p H[p,(p+tau)%1024] = S[(tau+128)%1024] where S is the
    column sum of the sheared buffer with its tail folded back in PSUM.
  - the partition reduce uses one-hot selector stationaries: psum row b =
    mv of batch b, row 4 = sum over batches, accumulated across batches.
  - AllReduce fires immediately after stats; Z matmuls fill the window.
  - aggregation: N_PE_AGG batches via weighted-identity PE accumulation,
    the rest via DVE MACs on dynamic windows of the doubled Z.
"""
import math
from contextlib import ExitStack
import numpy as np
import ml_dtypes

import concourse.bass as bass
import concourse.mybir as mybir
import concourse.tile as tile
from concourse import bacc
from concourse.bass import ds
from concourse.tile import TileContext
from concourse.bass_utils import run_bass_kernel_spmd

B, L, D = 32, 1024, 512
NCORES = 8
BLOC = B // NCORES          # 4 batches per core
TOPK = 6
F32 = mybir.dt.float32
BF16 = mybir.dt.bfloat16
U32 = mybir.dt.uint32
ALU = mybir.AluOpType
AFT = mybir.ActivationFunctionType

NJC = D // 128           # 4 chunks of output-feature rows
NIC = D // 128           # 4 chunks of contraction
NTC = L // 512           # 2 free-dim chunks of 512
NTR = L // 128           # 8 row-blocks of t'
N_PE_AGG = 3             # batches aggregated on the PE (rest on DVE)

_CACHE = {}


def _build():
    nc = bacc.Bacc("TRN2", target_bir_lowering=False)

    qT_d = nc.dram_tensor("qT", [BLOC, D, L], BF16, kind="ExternalInput")
    kT_d = nc.dram_tensor("kT", [BLOC, D, L], BF16, kind="ExternalInput")
    vT_d = nc.dram_tensor("vT", [BLOC, D, L], BF16, kind="ExternalInput")
    A_d = nc.dram_tensor("A", [D, D], BF16, kind="ExternalInput")
    Wc_d = nc.dram_tensor("Wc", [D, D], BF16, kind="ExternalInput")
    I_d = nc.dram_tensor("I128", [128, 128], BF16, kind="ExternalInput")
    sel_d = nc.dram_tensor("sel", [128, 4 * 8], BF16, kind="ExternalInput")
    bc_d = nc.dram_tensor("bcast", [4, 4 * 128], BF16, kind="ExternalInput")
    outT_d = nc.dram_tensor("outT", [BLOC, D, L], BF16, kind="ExternalOutput")
    cc_in = nc.dram_tensor("cc_in", [1, L], F32)
    cc_out = nc.dram_tensor("cc_out", [1, L], F32, addr_space="Shared")
    cc2_in = nc.dram_tensor("cc2_in", [1, L], F32)
    cc2_out = nc.dram_tensor("cc2_out", [1, L], F32, addr_space="Shared")

    with TileContext(nc) as tc, ExitStack() as ctx:
        consts = ctx.enter_context(tc.tile_pool(name="consts", bufs=1))
        in_pool = ctx.enter_context(tc.tile_pool(name="in", bufs=1))
        rho_pool = ctx.enter_context(tc.tile_pool(name="rho", bufs=2))
        pc_pool = ctx.enter_context(tc.tile_pool(name="pc", bufs=1))
        z_pool = ctx.enter_context(tc.tile_pool(name="z", bufs=4))
        gst_pool = ctx.enter_context(tc.tile_pool(name="gst", bufs=2))
        out_pool = ctx.enter_context(tc.tile_pool(name="out", bufs=4))
        small = ctx.enter_context(tc.tile_pool(name="small", bufs=1))

        # ---- constant tiles (DMAs emitted after the first qT/kT loads;
        #      Wc deferred to batch-1) ----
        A_sb = consts.tile([128, NIC, D], BF16)
        I_sb = consts.tile([128, 128], BF16)
        sel_sb = consts.tile([128, 4, 8], BF16)
        bc_sb = consts.tile([4, 4, 128], BF16)
        Wc_sb = consts.tile([128, NIC, D], BF16)

        # persistent sheared buffers (borders must stay zero across reuse)
        NPB = 2
        pbs = []
        for i in range(NPB):
            pb_t = pc_pool.tile([128, 1152], BF16, tag=f"pb{i}", name=f"pb{i}")
            nc.vector.memset(pb_t, 0.0)
            pbs.append(pb_t)

        # rows 0-3: per-batch mv (canonical tau order)
        mv_all = small.tile([4, L], F32)
        # bs rows live on partition 4 (aligned with mv_ps row 4; engines
        # cannot shift partitions, only DMA can)
        bs01_sb = small.tile([5, L], F32)
        bs23_sb = small.tile([5, L], F32)
        vT_tiles = {}
        qkT_tiles = {}

        # ============ phase 1: stats (rho + circulant Gram) ============
        ps1 = tc.alloc_tile_pool(name="ps1", bufs=1, space="PSUM")
        # rows b: mv of batch b; row 4: partial batch sums
        mv_ps = ps1.tile([8, L], F32, tag="mv", bufs=1)    # batches 0,1
        mv_ps2 = ps1.tile([8, L], F32, tag="mv2", bufs=1)  # batches 2,3

        def emit_qk_loads(b):
            qT = in_pool.tile([128, NIC, L], BF16, tag="xT", bufs=4,
                              name=f"qT{b}")
            if b == 0:
                # rho consumes (A[ic], qT0[ic]) pairs in ic order
                Ar = A_d.rearrange("(ic p) j -> p ic j", p=128)
                for ic in range(NIC):
                    nc.sync.dma_start(
                        out=qT[:, ic],
                        in_=qT_d.ap()[b, ic * 128:(ic + 1) * 128])
                    nc.sync.dma_start(out=A_sb[:, ic], in_=Ar[:, ic])
            else:
                for ic in range(NIC):
                    nc.sync.dma_start(
                        out=qT[:, ic],
                        in_=qT_d.ap()[b, ic * 128:(ic + 1) * 128])
            kT = in_pool.tile([128, NIC, L], BF16, tag="xT", bufs=4,
                              name=f"kT{b}")
            for ic in range(NIC):
                nc.sync.dma_start(
                    out=kT[:, ic], in_=kT_d.ap()[b, ic * 128:(ic + 1) * 128])
            if b == 0:
                nc.sync.dma_start(out=I_sb, in_=I_d.ap())
                nc.sync.dma_start(out=sel_sb, in_=sel_d.ap())
                nc.sync.dma_start(out=bc_sb, in_=bc_d.ap())
            qkT_tiles[b] = (qT, kT)

        def emit_vt(b):
            # vT is consumed in phase 2 only; emitted after the NEXT batch's
            # qT/kT so stats loads are never stuck behind it in the DMA queue
            vT = in_pool.tile([128, NIC, L], BF16, tag="vT", bufs=4,
                              name=f"vT{b}")
            nc.sync.dma_start(out=vT, in_=vT_d.ap()[b].rearrange(
                "(ic p) t -> p ic t", p=128))
            vT_tiles[b] = vT
            if b == 0:
                nc.sync.dma_start(
                    out=Wc_sb, in_=Wc_d.rearrange("(ic p) j -> p ic j", p=128))

        def emit_stats(b):
            qT, kT = qkT_tiles[b]
            # rhoT[j, t] = sum_i A[i, j] * qT[i, t]
            rho = rho_pool.tile([128, NJC, L], BF16, tag="rho", name=f"rho{b}")
            for jc in range(NJC):
                pss = [ps1.tile([128, 512], F32, tag="rps", bufs=2,
                                name=f"rps{b}_{jc}_{t}") for t in range(NTC)]
                for ic in range(NIC):
                    for tcc in range(NTC):
                        nc.tensor.matmul(
                            pss[tcc],
                            lhsT=A_sb[:, ic, jc * 128:(jc + 1) * 128],
                            rhs=qT[:, ic, tcc * 512:(tcc + 1) * 512],
                            start=(ic == 0), stop=(ic == NIC - 1))
                for tcc in range(NTC):
                    eng = (nc.scalar.copy if (jc + tcc) % 2 == 0
                           else nc.vector.tensor_copy)
                    eng(rho[:, jc, tcc * 512:(tcc + 1) * 512], pss[tcc])

            # H[p, c] = sum_{tr,j} kT[j, 128tr+p] * rho[j, (128tr+c) % L]
            H = ps1.tile([128, L], F32, tag="H", bufs=1, name=f"H{b}")
            for tr in range(NTR):
                base = 128 * tr
                for jc in range(NJC):
                    lhs = kT[:, jc, base:base + 128]
                    first = (tr == 0 and jc == 0)
                    last = (tr == NTR - 1 and jc == NJC - 1)
                    for w0 in (0, 512):
                        s = (base + w0) % L
                        if s + 512 <= L:
                            nc.tensor.matmul(
                                H[:, w0:w0 + 512], lhsT=lhs,
                                rhs=rho[:, jc, s:s + 512],
                                start=first, stop=last)
                        else:
                            w1 = L - s
                            nc.tensor.matmul(
                                H[:, w0:w0 + w1], lhsT=lhs,
                                rhs=rho[:, jc, s:L],
                                start=first, stop=last)
                            nc.tensor.matmul(
                                H[:, w0 + w1:w0 + 512], lhsT=lhs,
                                rhs=rho[:, jc, 0:512 - w1],
                                start=first, stop=last)

            # evac H (cast bf16), shear-spill: pb[p, 128 - p + c] = Hs[p, c]
            Hs = gst_pool.tile([128, L], BF16, tag="gst", name=f"Hs{b}")
            nc.scalar.copy(Hs[:, 0:512], H[:, 0:512])
            nc.vector.tensor_copy(Hs[:, 512:L], H[:, 512:L])
            pb = pbs[b % NPB]
            shear_a = bass.AP(tensor=pb.tensor, offset=pb.offset + 128,
                              ap=[[1152 - 1, 128], [1, 512]])
            nc.sync.dma_start(out=shear_a, in_=Hs[:, 0:512])
            shear_b = bass.AP(tensor=pb.tensor, offset=pb.offset + 128 + 512,
                              ap=[[1152 - 1, 128], [1, 512]])
            nc.sync.dma_start(out=shear_b, in_=Hs[:, 512:L])

        def emit_reduce(b):
            # ps[m, c] += sum_p sel[p, b, m] * pb[p, c], tail folded into
            # cols [0,128). start/stop flags are per-element: main matmuls
            # are first (start) / last (stop) writers of every column.
            ps = mv_ps if b < 2 else mv_ps2
            pb = pbs[b % NPB]
            sel = sel_sb[:, b, :]
            first = (b % 2 == 0)
            last = (b % 2 == 1)
            nc.tensor.matmul(ps[:, 0:512], lhsT=sel, rhs=pb[:, 0:512],
                             start=first, stop=last, skip_group_check=True)
            nc.tensor.matmul(ps[:, 0:128], lhsT=sel, rhs=pb[:, 1024:1152],
                             start=False, stop=last, skip_group_check=True)
            nc.tensor.matmul(ps[:, 512:1024], lhsT=sel, rhs=pb[:, 512:1024],
                             start=first, stop=last, skip_group_check=True)

        def emit_cc(tag, src, cin, cout):
            nc.sync.dma_start(out=cin.ap(), in_=src)
            nc.gpsimd.collective_compute(
                "AllReduce", ALU.add,
                replica_groups=[list(range(NCORES))],
                ins=[cin.ap()], outs=[cout.ap()])
            bmp = small.tile([1, L], F32, tag=tag, name=tag)
            nc.sync.dma_start(out=bmp, in_=cout.ap())
            return bmp

        emit_qk_loads(0)
        emit_stats(0)
        emit_qk_loads(1)
        emit_vt(0)
        emit_stats(1)
        emit_qk_loads(2)
        emit_vt(1)
        emit_reduce(0)
        emit_stats(2)
        emit_qk_loads(3)
        emit_vt(2)
        emit_reduce(1)
        # first collective: batches 0+1, hidden under stats of 2/3.
        # rotation to canonical: canonical[tau] = ps[(tau + 128) % 1024]
        nc.scalar.copy(bs01_sb[0:5, 0:896], mv_ps[0:5, 128:1024])
        nc.scalar.copy(bs01_sb[0:5, 896:L], mv_ps[0:5, 0:128])
        bm1 = emit_cc("bm1", bs01_sb[4:5, :], cc_in, cc_out)
        emit_reduce(2)
        emit_stats(3)
        emit_reduce(3)
        emit_vt(3)
        nc.scalar.copy(bs23_sb[0:5, 0:896], mv_ps2[0:5, 128:1024])
        nc.scalar.copy(bs23_sb[0:5, 896:L], mv_ps2[0:5, 0:128])
        bm2 = emit_cc("bm2", bs23_sb[4:5, :], cc2_in, cc2_out)
        # rows 2,3 of bs01 / rows 0,1 of bs23 are zeros (one-hot selectors),
        # so the per-batch mv rows are just the sum of the two staging tiles
        nc.vector.tensor_add(mv_all, bs01_sb[0:4, :], bs23_sb[0:4, :])
        ps1.release()

        # ============ phase 2: Z = Wc^T vT (+ doubled copy) ============
        ps2 = tc.alloc_tile_pool(name="ps2", bufs=1, space="PSUM")
        z_tiles = {}

        def emit_z(b):
            vT = vT_tiles[b]
            Z = z_pool.tile([128, NJC, 2 * L], BF16, tag="Z", name=f"Z{b}")
            for jc in range(NJC):
                pss = [ps2.tile([128, 512], F32, tag="zps", bufs=4,
                                name=f"zps{b}_{jc}_{t}") for t in range(NTC)]
                for ic in range(NIC):
                    for tcc in range(NTC):
                        nc.tensor.matmul(
                            pss[tcc],
                            lhsT=Wc_sb[:, ic, jc * 128:(jc + 1) * 128],
                            rhs=vT[:, ic, tcc * 512:(tcc + 1) * 512],
                            start=(ic == 0), stop=(ic == NIC - 1))
                # all Z evacs on scalar: keeps the vector queue free for
                # the weights chain (no head-of-line stall behind Z data)
                for tcc in range(NTC):
                    nc.scalar.copy(Z[:, jc, tcc * 512:(tcc + 1) * 512],
                                   pss[tcc])
                    nc.scalar.copy(Z[:, jc, L + tcc * 512:L + (tcc + 1) * 512],
                                   pss[tcc])
            z_tiles[b] = Z

        emit_z(0)
        emit_z(1)

        emit_z(2)
        emit_z(3)
        # ============ top-k + batched softmax weights ============
        # emitted after all Z work: every engine queue is already drained
        # of Z ops, so the chain runs purely on data-readiness (robust to
        # collective-latency variance).
        bm = small.tile([1, L], F32)
        nc.vector.tensor_add(bm, bm1, bm2)
        vals8 = small.tile([1, 8], F32)
        idx8 = small.tile([1, 8], U32)
        nc.vector.max_with_indices(vals8, idx8, bm)

        # gather registers on the vector engine (also reused for DVE agg)
        vregs = [nc.vector.alloc_register(f"tau_v{k}") for k in range(TOPK)]
        nc.vector.reg_load(vregs, idx8[0:1, 0:TOPK])
        tau_v = []
        for k in range(TOPK):
            lo = nc.snap(vregs[k], min_val=0, max_val=L - 1)
            r2 = nc.vector.alloc_register(f"tau_v{k}_hi")
            nc.vector.reg_add(r2, vregs[k], 512)
            hi = nc.snap(r2, min_val=512, max_val=L - 1 + 512)
            tau_v.append((lo, hi))

        wraw = small.tile([4, 8], F32)
        for k in range(TOPK):
            nc.vector.tensor_copy(wraw[0:4, k:k + 1],
                                  mv_all[0:4, ds(tau_v[k][0], 1)])
        w6 = wraw[0:4, 0:TOPK]
        mx = small.tile([4, 1], F32)
        nc.vector.tensor_reduce(out=mx, in_=w6, axis=mybir.AxisListType.X,
                                op=ALU.max)
        negmx = small.tile([4, 1], F32)
        nc.vector.tensor_scalar(out=negmx, in0=mx, scalar1=-1.0,
                                scalar2=None, op0=ALU.mult)
        ex = small.tile([4, 8], F32)
        sm = small.tile([4, 1], F32)
        nc.scalar.activation(ex[0:4, 0:TOPK], w6, AFT.Exp, bias=negmx,
                             accum_out=sm)
        rc = small.tile([4, 1], F32)
        nc.vector.reciprocal(rc, sm)
        wn = small.tile([4, 8], F32)
        nc.vector.tensor_scalar(out=wn[0:4, 0:TOPK], in0=ex[0:4, 0:TOPK],
                                scalar1=rc, scalar2=None, op0=ALU.mult)
        wnb = small.tile([4, 8], BF16)
        nc.vector.tensor_copy(wnb, wn)

        # broadcast weights to all 128 partitions: w_bc[:, b, k] = wn[b, k]
        w_ps = ps2.tile([128, 4, 8], F32, tag="wps", bufs=1)
        for b in range(BLOC):
            nc.tensor.matmul(w_ps[:, b, :], lhsT=bc_sb[0:4, b, :], rhs=wnb,
                             start=True, stop=True, skip_group_check=True)
        w_bc = small.tile([128, 4, 8], F32)
        nc.vector.tensor_copy(w_bc, w_ps)

        # weighted identities for the PE aggregation
        wIs = {}
        for b in range(N_PE_AGG):
            wi = consts.tile([128, TOPK, 128], BF16, tag=f"wI{b}",
                             name=f"wI{b}")
            for k in range(TOPK):
                nc.vector.tensor_scalar(out=wi[:, k, :], in0=I_sb,
                                        scalar1=w_bc[:, b, k:k + 1],
                                        scalar2=None, op0=ALU.mult)
            wIs[b] = wi


        # dynamic-offset registers for Z windows
        tregs = [nc.tensor.alloc_register(f"tau_t{k}") for k in range(TOPK)]
        nc.tensor.reg_load(tregs, idx8[0:1, 0:TOPK])
        tau_t = []
        for k in range(TOPK):
            lo = nc.snap(tregs[k], min_val=0, max_val=L - 1)
            r2 = nc.tensor.alloc_register(f"tau_t{k}_hi")
            nc.tensor.reg_add(r2, tregs[k], 512)
            hi = nc.snap(r2, min_val=512, max_val=L - 1 + 512)
            tau_t.append((lo, hi))
        ps2.release()

        # ============ phase 3: time-delay aggregation ============
        ps3 = tc.alloc_tile_pool(name="ps3", bufs=1, space="PSUM")

        def emit_agg_pe(b):
            # tile-major: finish one [128,512] output tile at a time so the
            # evac + output DMA stream throughout the batch instead of
            # bursting 2MB at the very end
            Z = z_tiles[b]
            wi = wIs[b]
            for jc in range(NJC):
                for tcc in range(NTC):
                    aps = ps3.tile([128, 512], F32, tag="aps", bufs=4,
                                   name=f"aps{b}_{jc}_{tcc}")
                    for k in range(TOPK):
                        nc.tensor.matmul(
                            aps,
                            lhsT=wi[:, k, :],
                            rhs=Z[:, jc, ds(tau_t[k][tcc], 512)],
                            start=(k == 0), stop=(k == TOPK - 1))
                    acc = out_pool.tile([128, 512], BF16, tag="acc",
                                        name=f"pacc{b}_{jc}_{tcc}")
                    nc.scalar.copy(acc, aps)
                    nc.sync.dma_start(
                        out=outT_d.ap()[b, jc * 128:(jc + 1) * 128,
                                        tcc * 512:(tcc + 1) * 512],
                        in_=acc)

        def emit_agg_dve(b):
            Z = z_tiles[b]
            for jc in range(NJC):
                for tcc in range(NTC):
                    accb = out_pool.tile([128, 512], BF16, tag="accb", bufs=2,
                                         name=f"accb{b}_{jc}_{tcc}")
                    nc.vector.tensor_scalar(
                        out=accb,
                        in0=Z[:, jc, ds(tau_v[0][tcc], 512)],
                        scalar1=w_bc[:, b, 0:1], scalar2=None, op0=ALU.mult)
                    for k in range(1, TOPK - 1):
                        nc.vector.scalar_tensor_tensor(
                            out=accb,
                            in0=Z[:, jc, ds(tau_v[k][tcc], 512)],
                            scalar=w_bc[:, b, k:k + 1],
                            in1=accb, op0=ALU.mult, op1=ALU.add)
                    acc = out_pool.tile([128, 512], BF16, tag="dacc", bufs=2,
                                        name=f"dacc{b}_{jc}_{tcc}")
                    nc.vector.scalar_tensor_tensor(
                        out=acc,
                        in0=Z[:, jc, ds(tau_v[TOPK - 1][tcc], 512)],
                        scalar=w_bc[:, b, TOPK - 1:TOPK],
                        in1=accb, op0=ALU.mult, op1=ALU.add)
                    nc.sync.dma_start(
                        out=outT_d.ap()[b, jc * 128:(jc + 1) * 128,
                                        tcc * 512:(tcc + 1) * 512],
                        in_=acc)

        # DVE batch first: its vector MACs start as soon as Z3 lands,
        # in parallel with all PE-agg matmuls
        for b in range(N_PE_AGG, BLOC):
            emit_agg_dve(b)
        for b in range(N_PE_AGG):
            emit_agg_pe(b)
        ps3.release()

    nc.compile()
    return nc


def _get_nc():
    if "nc" not in _CACHE:
        _CACHE["nc"] = _build()
    return _CACHE["nc"]


def _make_in_maps(inputs):
    bf16 = ml_dtypes.bfloat16
    q_in = np.asarray(inputs["q_in"], dtype=np.float32)
    k_in = np.asarray(inputs["k_in"], dtype=np.float32)
    v_in = np.asarray(inputs["v_in"], dtype=np.float32)
    Wq, Wk, Wv, Wo = (inputs["Wq"], inputs["Wk"], inputs["Wv"], inputs["Wo"])

    A = ((Wq.astype(np.float64) @ Wk.astype(np.float64).T) / D).astype(bf16)
    Wc = (Wv.astype(np.float64) @ Wo.astype(np.float64)).astype(bf16)

    qT = np.ascontiguousarray(q_in.transpose(0, 2, 1)).astype(bf16)
    kT = np.ascontiguousarray(k_in.transpose(0, 2, 1)).astype(bf16)
    vT = np.ascontiguousarray(v_in.transpose(0, 2, 1)).astype(bf16)

    sel = np.zeros((128, 4 * 8), dtype=bf16)
    for b in range(4):
        sel[:, b * 8 + b] = 1
        sel[:, b * 8 + 4] = 1
    bcast = np.zeros((4, 4 * 128), dtype=bf16)
    for b in range(4):
        bcast[b, b * 128:(b + 1) * 128] = 1

    in_maps = []
    for c in range(NCORES):
        sl = slice(c * BLOC, (c + 1) * BLOC)
        in_maps.append({
            "qT": qT[sl], "kT": kT[sl], "vT": vT[sl],
            "A": A, "Wc": Wc,
            "I128": np.eye(128, dtype=bf16),
            "sel": sel, "bcast": bcast,
        })
    return in_maps


def kernel(q_in, k_in, v_in, Wq, bq, Wk, bk, Wv, bv, Wo, bo):
    c_row = (bv.astype(np.float64) @ Wo.astype(np.float64) + bo).astype(np.float32)

    nc = _get_nc()
    in_maps = _make_in_maps({
        "q_in": q_in, "k_in": k_in, "v_in": v_in,
        "Wq": Wq, "Wk": Wk, "Wv": Wv, "Wo": Wo,
    })
    res = run_bass_kernel_spmd(nc, in_maps, list(range(NCORES)))
    outT = np.concatenate([np.asarray(r["outT"], dtype=np.float32)
                           for r in res.results], axis=0)  # (B, D, L)
    out = outT.transpose(0, 2, 1) + c_row[None, None, :]
    return np.ascontiguousarray(out, dtype=np.float32)
